# revision 17
# baseline (speedup 1.0000x reference)
"""Trainium2 Bass kernel for a full MHA block (QKV proj + softmax attention +
output proj + residual + LayerNorm), B=2, S=4096, E=512, H=8, D=64.

Sharding: sequence-parallel over 8 cores (4 seq shards x 2 batches). Each core
owns R=1024 query rows of one batch, recomputes K/V for the full context
(avoids all cross-core communication), and writes its own [R, E] output slice.

v2 layout/scheduling strategy (per core):
  - x^T pre-transposed+packed fp8 on host, t-blocked so DMA loads are
    contiguous 2KB/partition chunks spread over 4 engine queues
  - K^T/Q^T projections head-major [e_out/128, t] (fp8 DoubleRow, K=256)
  - wk pre-scaled by BITS_MUL on host so the DVE exp bit-trick is a
    single-op tensor_scalar add (2x mode) and scores arrive pre-scaled
  - scores: per t-tile the two heads of a pair are issued back-to-back at
    tile_position (0,0)/(64,0) so the K=64 matmuls run concurrently in
    separate PE row groups; A@V of the previous group follows them
  - exp split ~50/50 between ScalarE (true exp, scale folds the prescale)
    and DVE (log-domain fp8 bit trick)
  - A@V: lhsT = [V_h | ones] (80 cols, fp8 DoubleRow K=256); row 64
    accumulates the softmax denominator for free
  - normalize: batched reciprocal of the two denom rows, partition-broadcast
    on GpSimd, DVE multiply writing ctx^T directly in fp8 DoubleRow-packed
    layout for the O-projection
  - O-proj: 2 fp8 DoubleRow matmuls (K=256 each, all 8 heads) + residual
    (bo pre-folded into the residual input on host) + LayerNorm with
    rstd = exp(-0.5*ln(var+eps)) so ScalarE stays on one activation-table
    set (no Exp<->Sqrt table thrash)
"""

import sys

sys.path.insert(0, "/opt/trn_rl_repo")

import numpy as np
import ml_dtypes

import concourse.bass as bass
import concourse.bacc as bacc
import concourse.mybir as mybir
import concourse.tile as tile
from concourse.bass import ds, ts

# Problem constants (hardcoded per harness contract)
B = 2
S = 4096
E = 512
H = 8
D = 64
N_CORES = 8
SEQ_SHARDS = N_CORES // B
R = S // SEQ_SHARDS  # 1024 own query rows per core
G2 = E // 256        # DoubleRow chunk-pair groups for the projections

F32 = mybir.dt.float32
F16 = mybir.dt.float16
FP8 = mybir.dt.float8e4
VP = 80  # padded V columns (64 V + 1 ones + pad to a 16-multiple for DoubleRow)
EXP_SHIFT = -3.0  # exp(s/8 - 3): keeps exp outputs < fp8e4 max; cancels in softmax
# log-domain exp on DVE: fp8e4m3 bits of exp(s/8+SHIFT) == s*BITS_MUL + BITS_ADD,
# rounded + saturated to [0,255] by the uint8 convert (verified on HW).
# wk is pre-scaled by BITS_MUL on host, so scores arrive as s' = s*BITS_MUL and
# the DVE op is a single add; the scalar path divides the scale back out.
BITS_MUL = 11.5416529 / 8.0
BITS_ADD = 56.0 + 11.5416529 * EXP_SHIFT
SCALAR_SCALE = 1.0 / 11.5416529  # exp(s'/11.5416529 + SHIFT) == exp(s/8 + SHIFT)
AF = mybir.ActivationFunctionType
ALU = mybir.AluOpType


def build_mha(nc, seq=S, rows=R, zero_qk_bias=True, unit_ln=True,
              dve_num=4, dve_den=9):
    """Emit the Tile program. seq/rows shrinkable for simulation."""
    P = 128
    EC = E // P           # 4 e_out col blocks
    HPAIRS = H // 2       # 4 head-pair blocks (=e_out blocks of 128)
    TT = seq // P         # t tiles
    NPAIR = TT // 2       # t-tile pairs (DoubleRow A@V granularity)
    tblk = min(512, seq)
    TB = seq // tblk      # t blocks for xT DMA / K-proj
    tpb = tblk // P       # t tiles per block
    qblk = min(512, rows)
    QB = rows // qblk     # r blocks for Q-proj
    sblk = min(512, rows)
    SB = rows // sblk     # s blocks per core
    ST = rows // P        # s tiles for O-proj/LN
    NG = NPAIR            # score groups (one per t-tile pair)

    # ---- DRAM I/O ----
    # x fed pre-transposed+packed fp8 from host, t-blocked:
    #   xT[p, tb, g, i, u] = x[tb*tblk+u, (2g+i)*128+p]   (DoubleRow pair axis i)
    xT_d = nc.dram_tensor("xT_f8", [P, TB, G2, 2, tblk], FP8, kind="ExternalInput").ap()
    xoT_d = nc.dram_tensor("xoT_f8", [P, QB, G2, 2, qblk], FP8, kind="ExternalInput").ap()
    # residual rows with bo pre-folded on host
    xo_f32 = nc.dram_tensor("xo_f32", [rows, E], F32, kind="ExternalInput").ap()
    # weights pre-packed on host: wX[p, g, i, e] = w[(2g+i)*128+p, e]
    # (wk additionally pre-scaled by BITS_MUL)
    wq = nc.dram_tensor("wq_f8", [P, G2, 2, E], FP8, kind="ExternalInput").ap()
    wk = nc.dram_tensor("wk_f8", [P, G2, 2, E], FP8, kind="ExternalInput").ap()
    wv = nc.dram_tensor("wv_f8", [P, G2, 2, E], FP8, kind="ExternalInput").ap()
    wo = nc.dram_tensor("wo_f8", [P, G2, 2, E], FP8, kind="ExternalInput").ap()
    bv = nc.dram_tensor("bv", [E], F32, kind="ExternalInput").ap()
    if not zero_qk_bias:
        # host pre-scales bk by BITS_MUL to match the wk prescale
        bq = nc.dram_tensor("bq", [E], F32, kind="ExternalInput").ap()
        bk = nc.dram_tensor("bk", [E], F32, kind="ExternalInput").ap()
    if not unit_ln:
        ln_g = nc.dram_tensor("ln_g", [E], F32, kind="ExternalInput").ap()
        ln_b = nc.dram_tensor("ln_b", [E], F32, kind="ExternalInput").ap()
    y_out = nc.dram_tensor("y", [rows, E], F32, kind="ExternalOutput").ap()

    with tile.TileContext(nc) as tc:
        with (
            tc.tile_pool(name="singles", bufs=1) as singles,
            tc.tile_pool(name="kqv", bufs=1) as kqv,
            tc.tile_pool(name="vtiles", bufs=max(NPAIR, 2)) as vtiles,
            tc.tile_pool(name="at", bufs=4) as atp,
            tc.tile_pool(name="ctx", bufs=4) as ctxp,
            tc.tile_pool(name="norm", bufs=4) as normp,
            tc.tile_pool(name="yout", bufs=3) as youtp,
            tc.tile_pool(name="stg", bufs=2, space="PSUM") as stg,
            tc.tile_pool(name="acc", bufs=2, space="PSUM") as accp,
            tc.tile_pool(name="util", bufs=2, space="PSUM") as util,
        ):
            # ---------- weights / x^T loads (4 DMA queues, startup-critical
            # order: what emit_k(0,0)/emit_q(0,0)/emit_v(0..) need comes first)
            wq_sb = singles.tile([P, G2, 2, E], FP8, name="wq_sb")
            wk_sb = singles.tile([P, G2, 2, E], FP8, name="wk_sb")
            wv_sb = singles.tile([P, G2, 2, E], FP8, name="wv_sb")
            wo_sb = singles.tile([P, G2, 2, E], FP8, name="wo_sb")
            xT = singles.tile([P, TB, G2, 2, tblk], FP8, name="xT")
            xoT = singles.tile([P, QB, G2, 2, qblk], FP8, name="xoT")
            nc.sync.dma_start(xT[:, 0], xT_d[:, 0])
            nc.gpsimd.dma_start(wk_sb, wk)
            nc.scalar.dma_start(wq_sb, wq)
            nc.scalar.dma_start(xoT[:, 0], xoT_d[:, 0])
            nc.gpsimd.dma_start(wv_sb, wv)
            if TB > 1:
                nc.sync.dma_start(xT[:, 1], xT_d[:, 1])
            for rb in range(1, QB):
                nc.scalar.dma_start(xoT[:, rb], xoT_d[:, rb])
            qrot = [nc.sync, nc.gpsimd, nc.scalar]
            for tb in range(2, TB):
                qrot[tb % 3].dma_start(xT[:, tb], xT_d[:, tb])
            nc.gpsimd.dma_start(wo_sb, wo)

            # ---------- constants ----------
            bv_bc = singles.tile([P, E], F32, name="bv_bc")
            nc.gpsimd.dma_start(out=bv_bc, in_=bv[None, :].to_broadcast((P, E)))
            if not zero_qk_bias:
                bk_sb = singles.tile([P, EC], F32, name="bk_sb")
                bq_sb = singles.tile([P, EC], F32, name="bq_sb")
                nc.gpsimd.dma_start(bk_sb, bk.rearrange("(c p) -> p c", p=P))
                nc.gpsimd.dma_start(bq_sb, bq.rearrange("(c p) -> p c", p=P))
            if not unit_ln:
                g_bc = singles.tile([P, E], F32, name="g_bc")
                b_bc = singles.tile([P, E], F32, name="b_bc")
                nc.gpsimd.dma_start(out=g_bc, in_=ln_g[None, :].to_broadcast((P, E)))
                nc.gpsimd.dma_start(out=b_bc, in_=ln_b[None, :].to_broadcast((P, E)))
            shift_t = singles.tile([P, 1], F32, name="shift_t")
            nc.vector.memset(shift_t, EXP_SHIFT)
            # dense constant tiles: tensor_tensor is the only DVE op with a
            # fast PSUM-source path (tensor_scalar/copy run 1x from PSUM)
            bits_bc = singles.tile([P, 2, 512], F32, name="bits_bc")
            nc.vector.memset(bits_bc, BITS_ADD)
            zero_bc = singles.tile([P, 512], F32, name="zero_bc")
            nc.vector.memset(zero_bc, 0.0)
            # per-sb LayerNorm stats: bn_aggr lands mean/var pairs here so the
            # rsqrt can be batched on DVE (no ScalarE act-table thrash)
            mv8 = [singles.tile([P, 2 * max(ST // SB, 1)], F32, name=f"mv8_{sb}")
                   for sb in range(SB)]
            rstd8 = [singles.tile([P, max(ST // SB, 1)], F32, name=f"rstd8_{sb}")
                     for sb in range(SB)]
            # pre-warm the GpSimd ext-isa library for partition_broadcast (the
            # first call otherwise pays a ~7us IRAM library DMA mid-kernel)
            warm_in = singles.tile([1, 16], F32, name="warm_in")
            warm_out = singles.tile([D, 16], F32, name="warm_out")
            nc.vector.memset(warm_in, 1.0)
            nc.gpsimd.partition_broadcast(warm_out, warm_in)

            # ---------- projection targets ----------
            kT = [kqv.tile([P, seq], F16, name=f"kT_{hp}") for hp in range(HPAIRS)]
            qT = [kqv.tile([P, rows], F16, name=f"qT_{hp}") for hp in range(HPAIRS)]
            # ctx^T in fp8 DoubleRow-packed layout for the O-projection:
            # ctx_f8[g][p, i, s] = ctx[head=(256g+128i+p)//64, d=p%64, s] / denom
            ctx_f8 = [kqv.tile([P, 2, rows], FP8, name=f"ctxf8_{g}")
                      for g in range(G2)]

            # ---------- V projection (+bias, +ones col) ----------
            v_tiles = {}

            def emit_v(t):
                pair, i = divmod(t, 2)
                if i == 0:
                    vt = vtiles.tile([P, 2, H, VP], FP8, name=f"v_{pair}", tag="v")
                    nc.vector.memset(vt[:, :, :, D:VP], 0.0)
                    nc.vector.memset(vt[:, :, :, D : D + 1], 1.0)
                    v_tiles[pair] = vt
                vt = v_tiles[pair]
                ps = util.tile([P, E], F32, name="v_ps", tag="u")
                tb, u = divmod(t, tpb)
                for g in range(G2):
                    nc.tensor.matmul(
                        ps, lhsT=xT[:, tb, g, :, ds(u * P, P)], rhs=wv_sb[:, g, :, :],
                        start=(g == 0), stop=(g == G2 - 1),
                        perf_mode=mybir.MatmulPerfMode.DoubleRow,
                    )
                nc.vector.tensor_add(
                    out=vt[:, i, :, 0:D],
                    in0=ps.rearrange("p (h d) -> p h d", h=H),
                    in1=bv_bc.rearrange("p (h d) -> p h d", h=H),
                )

            # ---------- K^T / Q^T projections (per head-pair block) ----------
            def emit_k(hp, tb):
                ps = util.tile([P, 512], F32, name="k_ps", tag="u")
                for g in range(G2):
                    nc.tensor.matmul(
                        ps[:, :tblk], lhsT=wk_sb[:, g, :, ds(hp * P, P)],
                        rhs=xT[:, tb, g, :, :],
                        start=(g == 0), stop=(g == G2 - 1),
                        perf_mode=mybir.MatmulPerfMode.DoubleRow,
                    )
                dst = kT[hp][:, ds(tb * tblk, tblk)]
                if zero_qk_bias:
                    nc.vector.tensor_add(out=dst, in0=ps[:, :tblk],
                                         in1=zero_bc[:, :tblk])
                else:
                    nc.vector.tensor_scalar(
                        out=dst, in0=ps[:, :tblk],
                        scalar1=bk_sb[:, hp : hp + 1], scalar2=None,
                        op0=ALU.add,
                    )

            def emit_q(hp, rb):
                ps = util.tile([P, 512], F32, name="q_ps", tag="u")
                for g in range(G2):
                    nc.tensor.matmul(
                        ps[:, :qblk], lhsT=wq_sb[:, g, :, ds(hp * P, P)],
                        rhs=xoT[:, rb, g, :, :],
                        start=(g == 0), stop=(g == G2 - 1),
                        perf_mode=mybir.MatmulPerfMode.DoubleRow,
                    )
                dst = qT[hp][:, ds(rb * qblk, qblk)]
                if zero_qk_bias:
                    nc.vector.tensor_add(out=dst, in0=ps[:, :qblk],
                                         in1=zero_bc[:, :qblk])
                else:
                    nc.vector.tensor_scalar(
                        out=dst, in0=ps[:, :qblk],
                        scalar1=bq_sb[:, hp : hp + 1], scalar2=None,
                        op0=ALU.add,
                    )

            # ---------- attention ----------
            exp_ctr = [0]
            # deferred normalize closures (see v1): each block's tail runs a
            # few groups into the NEXT block so the PE queue never stalls on
            # the drain/reciprocal chain at block boundaries.
            pending_norm = []

            def attention(hp, sb, fillers_by_group, norm_first=False):
                if norm_first and pending_norm:
                    pending_norm.pop(0)()
                ctx_ps = [
                    accp.tile([VP, sblk], F32, name=f"ctx_{h}", tag="ctx")
                    for h in range(2)
                ]
                pending = []  # at-pairs awaiting A@V, one group behind

                def flush_av(last):
                    at_p, pair = pending.pop(0)
                    for h in range(2):
                        nc.tensor.matmul(
                            ctx_ps[h][:, :sblk],
                            lhsT=v_tiles[pair][:, :, hp * 2 + h, :],
                            rhs=at_p[h][:, :, :sblk],
                            start=(pair == 0), stop=last,
                            perf_mode=mybir.MatmulPerfMode.DoubleRow,
                        )

                consumed = set()
                for g in range(NG):
                    if g == 2 and not norm_first and pending_norm:
                        pending_norm.pop(0)()
                    st_pair = [
                        stg.tile([P, 2, 512], F32, name=f"stg_{h}", tag="stg")
                        for h in range(2)
                    ]
                    at_pair = [
                        atp.tile([P, 2, 512], FP8, name=f"at_{h}", tag="at")
                        for h in range(2)
                    ]
                    # A@V of the previous group first: its streams give the
                    # score LDWs time to prefetch into both row groups
                    if pending:
                        flush_av(False)
                    # scores: heads issued adjacently at row groups (0,0)/(64,0)
                    # so the two K=64 matmuls stream concurrently
                    for j in range(2):
                        t = 2 * g + j
                        for h in range(2):
                            nc.tensor.matmul(
                                st_pair[h][:, j, :sblk],
                                lhsT=kT[hp][ds(h * D, D), ts(t, P)],
                                rhs=qT[hp][ds(h * D, D), ds(sb * sblk, sblk)],
                                start=True, stop=True,
                                tile_position=(h * D, 0),
                            )
                    # exp: split between DVE (log-domain bit trick as a
                    # tensor_tensor add with a dense const tile -- the only op
                    # shape with a fast PSUM path; wk prescale folded the
                    # multiply) and ScalarE (true exp)
                    for h in range(2):
                        if (exp_ctr[0] * dve_num) % dve_den < dve_num:
                            nc.vector.tensor_tensor(
                                out=at_pair[h][:, :, :sblk].bitcast(mybir.dt.uint8),
                                in0=st_pair[h][:, :, :sblk],
                                in1=bits_bc[:, :, :sblk],
                                op=ALU.add,
                            )
                        else:
                            nc.scalar.activation(
                                out=at_pair[h][:, :, :sblk],
                                in_=st_pair[h][:, :, :sblk],
                                func=AF.Exp, scale=SCALAR_SCALE, bias=shift_t,
                            )
                        exp_ctr[0] += 1
                    pending.append((at_pair, g))
                    consumed.add(g)
                    for f in fillers_by_group.get(g, ()):
                        f()
                # run any fillers scheduled past the last group (small configs)
                for g_key in sorted(k for k in fillers_by_group if k not in consumed):
                    for f in fillers_by_group[g_key]:
                        f()
                flush_av(True)
                # drain ctx+denominator rows PSUM->SBUF on DVE (DMA cannot
                # touch PSUM); denom rows hop to partitions 0/1 by SBUF DMA
                ctx_sb = [ctxp.tile([D + 1, sblk], F32, name=f"cs_{h}", tag="cs")
                          for h in range(2)]
                den2 = normp.tile([2, sblk], F32, name="den2", tag="dn")
                for h in range(2):
                    nc.vector.tensor_copy(ctx_sb[h], ctx_ps[h][: D + 1, :sblk])
                    nc.sync.dma_start(den2[h : h + 1, :], ctx_sb[h][D : D + 1, :])

                def do_norm(hp=hp, sb=sb, ctx_sb=ctx_sb, den2=den2):
                    recip2 = normp.tile([2, sblk], F32, name="recip2", tag="rc")
                    nc.vector.reciprocal_approx_fast(out=recip2, in_=den2)
                    # partition_broadcast sources must sit at partition 0:
                    # hop row 1 down via SBUF DMA (off the critical path)
                    r1 = normp.tile([1, sblk], F32, name="recip_r1", tag="r1")
                    nc.gpsimd.dma_start(r1, recip2[1:2, :])
                    for h in range(2):
                        rb_t = normp.tile([D, sblk], F32, name=f"rb_{h}", tag="rb")
                        nc.gpsimd.partition_broadcast(
                            rb_t, recip2[0:1, :] if h == 0 else r1)
                        head = hp * 2 + h
                        gi, ii, plo = head // 4, (head % 4) // 2, D * (head % 2)
                        nc.vector.tensor_mul(
                            out=ctx_f8[gi][ds(plo, D), ii, ds(sb * sblk, sblk)],
                            in0=ctx_sb[h][0:D, :], in1=rb_t,
                        )

                pending_norm.append(do_norm)

            # ---------- O-projection + residual + LayerNorm ----------
            # split: head = O-proj + residual + bn stats (streamable during
            # attention); rsqrt = one batched DVE quake-rsqrt per sb (keeps
            # ScalarE on the exp table set -- no act-table thrash); tail =
            # (y-mu)*rstd apply + store.
            nst = max(ST // SB, 1)
            y_tiles = {}

            def emit_out_head(st):
                ps = util.tile([P, E], F32, name="o_ps", tag="u")
                for g in range(G2):
                    nc.tensor.matmul(
                        ps, lhsT=ctx_f8[g][:, :, ts(st, P)], rhs=wo_sb[:, g, :, :],
                        start=(g == 0), stop=(g == G2 - 1),
                        perf_mode=mybir.MatmulPerfMode.DoubleRow,
                    )
                xo_t = youtp.tile([P, E], F32, name="xo_t", tag="xo")
                nc.sync.dma_start(xo_t, xo_f32[ts(st, P), :])
                y_t = youtp.tile([P, E], F32, name=f"y_{st}", tag=f"y_{st}")
                nc.vector.tensor_add(out=y_t, in0=ps, in1=xo_t)
                y_tiles[st] = y_t
                stats = normp.tile([P, 6], F32, name="stats")
                nc.vector.bn_stats(out=stats, in_=y_t)
                sb, k = divmod(st, nst) if SB > 1 else (0, st)
                nc.vector.bn_aggr(out=mv8[sb][:, 2 * k : 2 * k + 2], in_=stats)

            # f32 whose bit pattern is the quake rsqrt magic 0x5f3759df
            qmagic = singles.tile([P, nst], F32, name="qmagic")
            nc.vector.memset(qmagic, 1.3211836172961054e19)

            def emit_rsqrt(sb):
                # rstd8[sb][:, k] = 1/sqrt(var_k + eps) via quake bit-trick +
                # 2 Newton steps, entirely on DVE over tiny [P, nst] tiles
                var = mv8[sb][:, 1 : 2 * nst : 2]
                v8 = normp.tile([P, nst], F32, name="q_v8", tag="qk")
                nc.vector.tensor_scalar(out=v8, in0=var, scalar1=1e-5,
                                        scalar2=None, op0=ALU.add)
                # y0 bits = magic - (v >> 1)
                sh = normp.tile([P, nst], mybir.dt.int32, name="q_sh", tag="qs")
                nc.vector.tensor_scalar(
                    out=sh, in0=v8.bitcast(mybir.dt.int32),
                    scalar1=1, scalar2=None, op0=ALU.logical_shift_right,
                )
                nc.vector.tensor_tensor(
                    out=sh, in0=qmagic.bitcast(mybir.dt.int32), in1=sh,
                    op=ALU.subtract,
                )
                y = sh.bitcast(F32)
                h_t = normp.tile([P, nst], F32, name="q_h", tag="qk2")
                nc.vector.tensor_scalar(out=h_t, in0=v8, scalar1=0.5,
                                        scalar2=None, op0=ALU.mult)
                for _ in range(2):
                    t_t = normp.tile([P, nst], F32, name="q_t", tag="qk3")
                    nc.vector.tensor_mul(out=t_t, in0=y, in1=y)
                    nc.vector.tensor_mul(out=t_t, in0=t_t, in1=h_t)
                    nc.vector.tensor_scalar(out=t_t, in0=t_t, scalar1=-1.0,
                                            scalar2=1.5, op0=ALU.mult,
                                            op1=ALU.add)
                    nc.vector.tensor_mul(out=rstd8[sb], in0=y, in1=t_t)
                    y = rstd8[sb]

            def emit_out_tail(st):
                y_t = y_tiles.pop(st)
                sb, k = divmod(st, nst) if SB > 1 else (0, st)
                nc.vector.tensor_scalar(
                    out=y_t, in0=y_t,
                    scalar1=mv8[sb][:, 2 * k : 2 * k + 1],
                    scalar2=rstd8[sb][:, k : k + 1],
                    op0=ALU.subtract, op1=ALU.mult,
                )
                if not unit_ln:
                    nc.vector.tensor_mul(out=y_t, in0=y_t, in1=g_bc)
                    nc.vector.tensor_add(out=y_t, in0=y_t, in1=b_bc)
                nc.sync.dma_start(y_out[ts(st, P), :], y_t)


            # ---------- emission: sb-major; projections stream as fillers ----
            # prologue: just enough for attention(0, sb0) to start
            emit_k(0, 0)
            emit_q(0, 0)
            for t in range(min(4, TT)):
                emit_v(t)
            if TB > 1:
                emit_k(0, 1)

            def sched(items, ng):
                """Spread callables over groups [0, ng): dict g -> [fns]."""
                by_g = {}
                if not items:
                    return by_g
                per = max(1, (len(items) + ng - 1) // ng)
                it = iter(items)
                for g in range(ng):
                    chunk = []
                    for _ in range(per):
                        f = next(it, None)
                        if f is None:
                            break
                        chunk.append(f)
                    if chunk:
                        by_g[g] = chunk
                    else:
                        break
                return by_g

            emitted_out = set()

            def of(st):
                def run():
                    emit_out_head(st)
                    emitted_out.add(st)
                return run

            for sb in range(SB):
                for hp in range(HPAIRS):
                    fb = {}
                    if sb == 0:
                        if hp == 0:
                            # self-stream: rest of own kT two groups ahead,
                            # V pairs two pairs ahead
                            for g in range(NG):
                                fs = []
                                if g % 2 == 0 and 2 <= g // 2 + 2 < TB:
                                    fs.append(lambda tb=g // 2 + 2: emit_k(0, tb))
                                p = g + 2
                                if 2 <= p < NPAIR:
                                    fs.append(lambda t=2 * p: emit_v(t))
                                    fs.append(lambda t=2 * p + 1: emit_v(t))
                                if fs:
                                    fb[g] = fs
                            # next head-pair's first k-blocks + q at the tail
                            tail = []
                            if HPAIRS > 1:
                                for tb in range(min(2, TB)):
                                    tail.append(lambda tb=tb: emit_k(1, tb))
                                tail.append(lambda: emit_q(1, 0))
                            for i, f in enumerate(tail):
                                fb.setdefault(max(0, NG - 3) + i % 3, []).append(f)
                        else:
                            items = []
                            for tb in range(2, TB):
                                items.append(lambda hp=hp, tb=tb: emit_k(hp, tb))
                            if hp + 1 < HPAIRS:
                                for tb in range(min(2, TB)):
                                    items.append(
                                        lambda hp=hp + 1, tb=tb: emit_k(hp, tb))
                                items.append(lambda hp=hp + 1: emit_q(hp, 0))
                            elif SB > 1:
                                for h2 in range(HPAIRS):
                                    items.append(lambda h2=h2: emit_q(h2, 1))
                            fb = sched(items, NG)
                    else:
                        # sb1 pass: stream one sb0 output head per block,
                        # after the deferred norms have landed (g >= 4); the
                        # batched rsqrt + applies ride the last block
                        outs_per_block = (ST // SB + HPAIRS - 1) // HPAIRS
                        items = []
                        for k in range(outs_per_block):
                            st = hp * outs_per_block + k
                            if st < ST // SB:
                                items.append(of(st))
                        for i, f in enumerate(items):
                            fb.setdefault(min(4 + i, NG - 1), []).append(f)
                        if hp == HPAIRS - 1:
                            fb.setdefault(min(8, NG - 1), []).append(
                                lambda: emit_rsqrt(0))
                            for k in range(ST // SB):
                                fb.setdefault(min(10 + k, NG - 1), []).append(
                                    lambda st=k: emit_out_tail(st))
                    attention(hp, sb, fb,
                              norm_first=(sb > 0 and hp == 0 and SB > 1))

            while pending_norm:
                pending_norm.pop(0)()
            for st in range(ST):
                if st not in emitted_out:
                    emit_out_head(st)
            emit_rsqrt(SB - 1)
            for st in range((SB - 1) * nst, ST):
                emit_out_tail(st)

    return nc


_CACHED = {}


def _get_nc(seq=S, rows=R, zero_qk_bias=True, unit_ln=True, dve_num=4, dve_den=9):
    key = (seq, rows, zero_qk_bias, unit_ln, dve_num, dve_den)
    if key not in _CACHED:
        nc = bacc.Bacc("TRN2", target_bir_lowering=False, debug=False,
                       num_devices=N_CORES)
        build_mha(nc, seq=seq, rows=rows, zero_qk_bias=zero_qk_bias,
                  unit_ln=unit_ln, dve_num=dve_num, dve_den=dve_den)
        nc.compile()
        _CACHED[key] = nc
    return _CACHED[key]


def pack_fp8_tb(x2d, tblk=512):
    """[S, E] f32 -> [128, TB, G2, 2, tblk] fp8 with
    out[p, tb, g, i, u] = x[tb*tblk+u, (2g+i)*128+p]."""
    f8 = ml_dtypes.float8_e4m3
    s, e = x2d.shape
    tb = s // tblk
    # x.T [E, S] -> [G2, 2, 128, TB, tblk] -> [128, TB, G2, 2, tblk]
    return np.ascontiguousarray(
        np.asarray(x2d, np.float32).T
        .reshape(e // 256, 2, 128, tb, tblk)
        .transpose(2, 3, 0, 1, 4)
        .astype(f8)
    )


def packw_fp8_dr(w, scale=1.0):
    """[E, E] f32 -> [128, G2, 2, E] fp8 with out[p, g, i, e] = w[(2g+i)*128+p, e]."""
    f8 = ml_dtypes.float8_e4m3
    e_in, e_out = w.shape
    return np.ascontiguousarray(
        (np.asarray(w, np.float32) * scale)
        .reshape(e_in // 256, 2, 128, e_out)
        .transpose(2, 0, 1, 3)
        .astype(f8)
    )


def make_in_maps(inputs, zero_qk_bias, unit_ln):
    """Shard full inputs into per-core input dicts."""
    x = np.asarray(inputs["x"], np.float32)
    bo = np.asarray(inputs["bo"], np.float32)
    shared = {
        "wq_f8": packw_fp8_dr(inputs["wq"]),
        "wk_f8": packw_fp8_dr(inputs["wk"], scale=BITS_MUL),
        "wv_f8": packw_fp8_dr(inputs["wv"]),
        "wo_f8": packw_fp8_dr(inputs["wo"]),
        "bv": np.asarray(inputs["bv"], np.float32),
    }
    if not zero_qk_bias:
        shared["bq"] = np.asarray(inputs["bq"], np.float32)
        shared["bk"] = np.asarray(inputs["bk"], np.float32) * BITS_MUL
    if not unit_ln:
        shared["ln_g"] = np.asarray(inputs["ln_g"], np.float32)
        shared["ln_b"] = np.asarray(inputs["ln_b"], np.float32)
    xT_all = [pack_fp8_tb(x[b]) for b in range(B)]
    in_maps = []
    for c in range(N_CORES):
        b, shard = divmod(c, SEQ_SHARDS)
        r0 = shard * R
        m = dict(shared)
        m["xT_f8"] = xT_all[b]
        m["xoT_f8"] = pack_fp8_tb(x[b, r0 : r0 + R])
        m["xo_f32"] = np.ascontiguousarray(x[b, r0 : r0 + R] + bo)
        in_maps.append(m)
    return in_maps


def kernel(**inputs):
    from concourse import bass_utils

    zero_qk_bias = (not np.any(inputs["bq"])) and (not np.any(inputs["bk"]))
    unit_ln = bool(np.all(np.asarray(inputs["ln_g"]) == 1.0)) and (
        not np.any(inputs["ln_b"]))
    nc = _get_nc(zero_qk_bias=zero_qk_bias, unit_ln=unit_ln)
    in_maps = make_in_maps(inputs, zero_qk_bias, unit_ln)
    res = bass_utils.run_bass_kernel_spmd(nc, in_maps, core_ids=list(range(N_CORES)))
    out = np.empty((B, S, E), np.float32)
    for c in range(N_CORES):
        b, shard = divmod(c, SEQ_SHARDS)
        out[b, shard * R : (shard + 1) * R] = res.results[c]["y"]
    return out


# revision 19
# speedup vs baseline: 1.1546x; 1.1546x over previous
"""Trainium2 Bass kernel for a full MHA block (QKV proj + softmax attention +
output proj + residual + LayerNorm), B=2, S=4096, E=512, H=8, D=64.

Sharding: sequence-parallel over 8 cores (4 seq shards x 2 batches). Each core
owns R=1024 query rows of one batch, recomputes K/V for the full context
(avoids all cross-core communication), and writes its own [R, E] output slice.

v2 layout/scheduling strategy (per core):
  - x^T pre-transposed+packed fp8 on host, t-blocked so DMA loads are
    contiguous 2KB/partition chunks spread over 4 engine queues
  - K^T/Q^T projections head-major [e_out/128, t] (fp8 DoubleRow, K=256)
  - wk pre-scaled by BITS_MUL on host so the DVE exp bit-trick is a
    single-op tensor_scalar add (2x mode) and scores arrive pre-scaled
  - scores: per t-tile the two heads of a pair are issued back-to-back at
    tile_position (0,0)/(64,0) so the K=64 matmuls run concurrently in
    separate PE row groups; A@V of the previous group follows them
  - exp split ~50/50 between ScalarE (true exp, scale folds the prescale)
    and DVE (log-domain fp8 bit trick)
  - A@V: lhsT = [V_h | ones] (80 cols, fp8 DoubleRow K=256); row 64
    accumulates the softmax denominator for free
  - normalize: batched reciprocal of the two denom rows, partition-broadcast
    on GpSimd, DVE multiply writing ctx^T directly in fp8 DoubleRow-packed
    layout for the O-projection
  - O-proj: 2 fp8 DoubleRow matmuls (K=256 each, all 8 heads) + residual
    (bo pre-folded into the residual input on host) + LayerNorm with
    rstd = exp(-0.5*ln(var+eps)) so ScalarE stays on one activation-table
    set (no Exp<->Sqrt table thrash)
"""

import sys

sys.path.insert(0, "/opt/trn_rl_repo")

import numpy as np
import ml_dtypes

import concourse.bass as bass
import concourse.bacc as bacc
import concourse.mybir as mybir
import concourse.tile as tile
from concourse.bass import ds, ts

# Problem constants (hardcoded per harness contract)
B = 2
S = 4096
E = 512
H = 8
D = 64
N_CORES = 8
SEQ_SHARDS = N_CORES // B
R = S // SEQ_SHARDS  # 1024 own query rows per core
G2 = E // 256        # DoubleRow chunk-pair groups for the projections

F32 = mybir.dt.float32
F16 = mybir.dt.float16
FP8 = mybir.dt.float8e4
VP = 80  # padded V columns (64 V + 1 ones + pad to a 16-multiple for DoubleRow)
EXP_SHIFT = -3.0  # exp(s/8 - 3): keeps exp outputs < fp8e4 max; cancels in softmax
# log-domain exp on DVE: fp8e4m3 bits of exp(s/8+SHIFT) == s*BITS_MUL + BITS_ADD,
# rounded + saturated to [0,255] by the uint8 convert (verified on HW).
# wk is pre-scaled by BITS_MUL on host, so scores arrive as s' = s*BITS_MUL and
# the DVE op is a single add; the scalar path divides the scale back out.
BITS_MUL = 11.5416529 / 8.0
BITS_ADD = 56.0 + 11.5416529 * EXP_SHIFT
SCALAR_SCALE = 1.0 / 11.5416529  # exp(s'/11.5416529 + SHIFT) == exp(s/8 + SHIFT)
AF = mybir.ActivationFunctionType
ALU = mybir.AluOpType


def build_mha(nc, seq=S, rows=R, zero_qk_bias=True, unit_ln=True,
              dve_num=1, dve_den=3):
    """Emit the Tile program. seq/rows shrinkable for simulation."""
    P = 128
    EC = E // P           # 4 e_out col blocks
    HPAIRS = H // 2       # 4 head-pair blocks (=e_out blocks of 128)
    TT = seq // P         # t tiles
    NPAIR = TT // 2       # t-tile pairs (DoubleRow A@V granularity)
    tblk = min(512, seq)
    TB = seq // tblk      # t blocks for xT DMA / K-proj
    tpb = tblk // P       # t tiles per block
    qblk = min(512, rows)
    QB = rows // qblk     # r blocks for Q-proj
    sblk = min(512, rows)
    SB = rows // sblk     # s blocks per core
    ST = rows // P        # s tiles for O-proj/LN
    NG = NPAIR            # score groups (one per t-tile pair)

    # ---- DRAM I/O ----
    # x fed pre-transposed+packed fp8 from host, t-blocked:
    #   xT[p, tb, g, i, u] = x[tb*tblk+u, (2g+i)*128+p]   (DoubleRow pair axis i)
    xT_d = nc.dram_tensor("xT_f8", [P, TB, G2, 2, tblk], FP8, kind="ExternalInput").ap()
    xoT_d = nc.dram_tensor("xoT_f8", [P, QB, G2, 2, qblk], FP8, kind="ExternalInput").ap()
    # residual rows with bo pre-folded on host
    xo_f32 = nc.dram_tensor("xo_f32", [rows, E], F32, kind="ExternalInput").ap()
    # weights pre-packed on host: wX[p, g, i, e] = w[(2g+i)*128+p, e]
    # (wk additionally pre-scaled by BITS_MUL)
    wq = nc.dram_tensor("wq_f8", [P, G2, 2, E], FP8, kind="ExternalInput").ap()
    wk = nc.dram_tensor("wk_f8", [P, G2, 2, E], FP8, kind="ExternalInput").ap()
    wv = nc.dram_tensor("wv_f8", [P, G2, 2, E], FP8, kind="ExternalInput").ap()
    wo = nc.dram_tensor("wo_f8", [P, G2, 2, E], FP8, kind="ExternalInput").ap()
    bv = nc.dram_tensor("bv", [E], F32, kind="ExternalInput").ap()
    if not zero_qk_bias:
        # host pre-scales bk by BITS_MUL to match the wk prescale
        bq = nc.dram_tensor("bq", [E], F32, kind="ExternalInput").ap()
        bk = nc.dram_tensor("bk", [E], F32, kind="ExternalInput").ap()
    if not unit_ln:
        ln_g = nc.dram_tensor("ln_g", [E], F32, kind="ExternalInput").ap()
        ln_b = nc.dram_tensor("ln_b", [E], F32, kind="ExternalInput").ap()
    y_out = nc.dram_tensor("y", [rows, E], F32, kind="ExternalOutput").ap()

    with tile.TileContext(nc) as tc:
        with (
            tc.tile_pool(name="singles", bufs=1) as singles,
            tc.tile_pool(name="kqv", bufs=1) as kqv,
            tc.tile_pool(name="vtiles", bufs=max(NPAIR, 2)) as vtiles,
            tc.tile_pool(name="at", bufs=4) as atp,
            tc.tile_pool(name="ctx", bufs=4) as ctxp,
            tc.tile_pool(name="norm", bufs=4) as normp,
            tc.tile_pool(name="yout", bufs=3) as youtp,
            tc.tile_pool(name="stg", bufs=2, space="PSUM") as stg,
            tc.tile_pool(name="acc", bufs=2, space="PSUM") as accp,
            tc.tile_pool(name="util", bufs=2, space="PSUM") as util,
        ):
            # ---------- weights / x^T loads (4 DMA queues, startup-critical
            # order: what emit_k(0,0)/emit_q(0,0)/emit_v(0..) need comes first)
            wq_sb = singles.tile([P, G2, 2, E], FP8, name="wq_sb")
            wk_sb = singles.tile([P, G2, 2, E], FP8, name="wk_sb")
            wv_sb = singles.tile([P, G2, 2, E], FP8, name="wv_sb")
            wo_sb = singles.tile([P, G2, 2, E], FP8, name="wo_sb")
            xT = singles.tile([P, TB, G2, 2, tblk], FP8, name="xT")
            xoT = singles.tile([P, QB, G2, 2, qblk], FP8, name="xoT")
            nc.sync.dma_start(xT[:, 0], xT_d[:, 0])
            nc.gpsimd.dma_start(wk_sb, wk)
            nc.scalar.dma_start(wq_sb, wq)
            nc.scalar.dma_start(xoT[:, 0], xoT_d[:, 0])
            nc.gpsimd.dma_start(wv_sb, wv)
            if TB > 1:
                nc.sync.dma_start(xT[:, 1], xT_d[:, 1])
            for rb in range(1, QB):
                nc.scalar.dma_start(xoT[:, rb], xoT_d[:, rb])
            qrot = [nc.sync, nc.gpsimd, nc.scalar]
            for tb in range(2, TB):
                qrot[tb % 3].dma_start(xT[:, tb], xT_d[:, tb])
            nc.gpsimd.dma_start(wo_sb, wo)

            # ---------- constants ----------
            bv_bc = singles.tile([P, E], F32, name="bv_bc")
            nc.gpsimd.dma_start(out=bv_bc, in_=bv[None, :].to_broadcast((P, E)))
            if not zero_qk_bias:
                bk_sb = singles.tile([P, EC], F32, name="bk_sb")
                bq_sb = singles.tile([P, EC], F32, name="bq_sb")
                nc.gpsimd.dma_start(bk_sb, bk.rearrange("(c p) -> p c", p=P))
                nc.gpsimd.dma_start(bq_sb, bq.rearrange("(c p) -> p c", p=P))
            if not unit_ln:
                g_bc = singles.tile([P, E], F32, name="g_bc")
                b_bc = singles.tile([P, E], F32, name="b_bc")
                nc.gpsimd.dma_start(out=g_bc, in_=ln_g[None, :].to_broadcast((P, E)))
                nc.gpsimd.dma_start(out=b_bc, in_=ln_b[None, :].to_broadcast((P, E)))
            shift_t = singles.tile([P, 1], F32, name="shift_t")
            nc.vector.memset(shift_t, EXP_SHIFT)
            # per-sb LayerNorm stats: bn_aggr lands mean/var pairs here so the
            # rsqrt can be batched on DVE (no ScalarE act-table thrash)
            mv8 = [singles.tile([P, 2 * max(ST // SB, 1)], F32, name=f"mv8_{sb}")
                   for sb in range(SB)]
            rstd8 = [singles.tile([P, max(ST // SB, 1)], F32, name=f"rstd8_{sb}")
                     for sb in range(SB)]
            # pre-warm the GpSimd ext-isa library for partition_broadcast (the
            # first call otherwise pays a ~7us IRAM library DMA mid-kernel)
            warm_in = singles.tile([1, 16], F32, name="warm_in")
            warm_out = singles.tile([D, 16], F32, name="warm_out")
            nc.vector.memset(warm_in, 1.0)
            nc.gpsimd.partition_broadcast(warm_out, warm_in)

            # ---------- projection targets ----------
            kT = [kqv.tile([P, seq], F16, name=f"kT_{hp}") for hp in range(HPAIRS)]
            qT = [kqv.tile([P, rows], F16, name=f"qT_{hp}") for hp in range(HPAIRS)]
            # ctx^T in fp8 DoubleRow-packed layout for the O-projection:
            # ctx_f8[g][p, i, s] = ctx[head=(256g+128i+p)//64, d=p%64, s] / denom
            ctx_f8 = [kqv.tile([P, 2, rows], FP8, name=f"ctxf8_{g}")
                      for g in range(G2)]

            # ---------- V projection (+bias, +ones col) ----------
            v_tiles = {}

            def emit_v(t):
                pair, i = divmod(t, 2)
                if i == 0:
                    vt = vtiles.tile([P, 2, H, VP], FP8, name=f"v_{pair}", tag="v")
                    nc.vector.memset(vt[:, :, :, D:VP], 0.0)
                    nc.vector.memset(vt[:, :, :, D : D + 1], 1.0)
                    v_tiles[pair] = vt
                vt = v_tiles[pair]
                ps = util.tile([P, E], F32, name="v_ps", tag="u")
                tb, u = divmod(t, tpb)
                for g in range(G2):
                    nc.tensor.matmul(
                        ps, lhsT=xT[:, tb, g, :, ds(u * P, P)], rhs=wv_sb[:, g, :, :],
                        start=(g == 0), stop=(g == G2 - 1),
                        perf_mode=mybir.MatmulPerfMode.DoubleRow,
                    )
                nc.vector.tensor_add(
                    out=vt[:, i, :, 0:D],
                    in0=ps.rearrange("p (h d) -> p h d", h=H),
                    in1=bv_bc.rearrange("p (h d) -> p h d", h=H),
                )

            # ---------- K^T / Q^T projections (per head-pair block) ----------
            def emit_k(hp, tb):
                ps = util.tile([P, 512], F32, name="k_ps", tag="u")
                for g in range(G2):
                    nc.tensor.matmul(
                        ps[:, :tblk], lhsT=wk_sb[:, g, :, ds(hp * P, P)],
                        rhs=xT[:, tb, g, :, :],
                        start=(g == 0), stop=(g == G2 - 1),
                        perf_mode=mybir.MatmulPerfMode.DoubleRow,
                    )
                dst = kT[hp][:, ds(tb * tblk, tblk)]
                if zero_qk_bias:
                    nc.vector.tensor_copy(dst, ps[:, :tblk])
                else:
                    nc.vector.tensor_scalar(
                        out=dst, in0=ps[:, :tblk],
                        scalar1=bk_sb[:, hp : hp + 1], scalar2=None,
                        op0=ALU.add,
                    )

            def emit_q(hp, rb):
                ps = util.tile([P, 512], F32, name="q_ps", tag="u")
                for g in range(G2):
                    nc.tensor.matmul(
                        ps[:, :qblk], lhsT=wq_sb[:, g, :, ds(hp * P, P)],
                        rhs=xoT[:, rb, g, :, :],
                        start=(g == 0), stop=(g == G2 - 1),
                        perf_mode=mybir.MatmulPerfMode.DoubleRow,
                    )
                dst = qT[hp][:, ds(rb * qblk, qblk)]
                if zero_qk_bias:
                    nc.vector.tensor_copy(dst, ps[:, :qblk])
                else:
                    nc.vector.tensor_scalar(
                        out=dst, in0=ps[:, :qblk],
                        scalar1=bq_sb[:, hp : hp + 1], scalar2=None,
                        op0=ALU.add,
                    )

            # ---------- attention ----------
            exp_ctr = [0]
            # deferred normalize closures (see v1): each block's tail runs a
            # few groups into the NEXT block so the PE queue never stalls on
            # the drain/reciprocal chain at block boundaries.
            pending_norm = []

            def attention(hp, sb, fillers_by_group, norm_first=False):
                if norm_first and pending_norm:
                    pending_norm.pop(0)()
                ctx_ps = [
                    accp.tile([VP, sblk], F32, name=f"ctx_{h}", tag="ctx")
                    for h in range(2)
                ]
                pending = []  # at-pairs awaiting A@V, one group behind

                def flush_av(last):
                    at_p, pair = pending.pop(0)
                    for h in range(2):
                        nc.tensor.matmul(
                            ctx_ps[h][:, :sblk],
                            lhsT=v_tiles[pair][:, :, hp * 2 + h, :],
                            rhs=at_p[h][:, :, :sblk],
                            start=(pair == 0), stop=last,
                            perf_mode=mybir.MatmulPerfMode.DoubleRow,
                        )

                consumed = set()
                for g in range(NG):
                    if g == 2 and not norm_first and pending_norm:
                        pending_norm.pop(0)()
                    st_pair = [
                        stg.tile([P, 2, 512], F32, name=f"stg_{h}", tag="stg")
                        for h in range(2)
                    ]
                    at_pair = [
                        atp.tile([P, 2, 512], FP8, name=f"at_{h}", tag="at")
                        for h in range(2)
                    ]
                    # scores: heads issued adjacently at row groups (0,0)/(64,0)
                    # so the two K=64 matmuls stream concurrently
                    for j in range(2):
                        t = 2 * g + j
                        for h in range(2):
                            nc.tensor.matmul(
                                st_pair[h][:, j, :sblk],
                                lhsT=kT[hp][ds(h * D, D), ts(t, P)],
                                rhs=qT[hp][ds(h * D, D), ds(sb * sblk, sblk)],
                                start=True, stop=True,
                                tile_position=(h * D, 0),
                            )
                    # A@V of the previous group runs while exp(g) processes
                    if pending:
                        flush_av(False)
                    # exp: split between DVE (log-domain bit trick: single add,
                    # wk prescale folded the multiply) and ScalarE (true exp);
                    # both are capped ~1x by the PSUM read port
                    for h in range(2):
                        if (exp_ctr[0] * dve_num) % dve_den < dve_num:
                            nc.vector.tensor_scalar(
                                out=at_pair[h][:, :, :sblk].bitcast(mybir.dt.uint8),
                                in0=st_pair[h][:, :, :sblk],
                                scalar1=BITS_ADD, scalar2=None,
                                op0=ALU.add,
                            )
                        else:
                            nc.scalar.activation(
                                out=at_pair[h][:, :, :sblk],
                                in_=st_pair[h][:, :, :sblk],
                                func=AF.Exp, scale=SCALAR_SCALE, bias=shift_t,
                            )
                        exp_ctr[0] += 1
                    pending.append((at_pair, g))
                    consumed.add(g)
                    for f in fillers_by_group.get(g, ()):
                        f()
                # run any fillers scheduled past the last group (small configs)
                for g_key in sorted(k for k in fillers_by_group if k not in consumed):
                    for f in fillers_by_group[g_key]:
                        f()
                flush_av(True)
                # drain ctx+denominator rows PSUM->SBUF on DVE (DMA cannot
                # touch PSUM); denom rows hop to partitions 0/1 by SBUF DMA
                ctx_sb = [ctxp.tile([D + 1, sblk], F32, name=f"cs_{h}", tag="cs")
                          for h in range(2)]
                den2 = normp.tile([2, sblk], F32, name="den2", tag="dn")
                for h in range(2):
                    nc.vector.tensor_copy(ctx_sb[h], ctx_ps[h][: D + 1, :sblk])
                    nc.sync.dma_start(den2[h : h + 1, :], ctx_sb[h][D : D + 1, :])

                def do_norm(hp=hp, sb=sb, ctx_sb=ctx_sb, den2=den2):
                    recip2 = normp.tile([2, sblk], F32, name="recip2", tag="rc")
                    nc.vector.reciprocal_approx_fast(out=recip2, in_=den2)
                    # partition_broadcast sources must sit at partition 0:
                    # hop row 1 down via SBUF DMA (off the critical path)
                    r1 = normp.tile([1, sblk], F32, name="recip_r1", tag="r1")
                    nc.gpsimd.dma_start(r1, recip2[1:2, :])
                    for h in range(2):
                        rb_t = normp.tile([D, sblk], F32, name=f"rb_{h}", tag="rb")
                        nc.gpsimd.partition_broadcast(
                            rb_t, recip2[0:1, :] if h == 0 else r1)
                        head = hp * 2 + h
                        gi, ii, plo = head // 4, (head % 4) // 2, D * (head % 2)
                        nc.vector.tensor_mul(
                            out=ctx_f8[gi][ds(plo, D), ii, ds(sb * sblk, sblk)],
                            in0=ctx_sb[h][0:D, :], in1=rb_t,
                        )

                pending_norm.append(do_norm)

            # ---------- O-projection + residual + LayerNorm ----------
            # split: head = O-proj + residual + bn stats (streamable during
            # attention); rsqrt = one batched DVE quake-rsqrt per sb (keeps
            # ScalarE on the exp table set -- no act-table thrash); tail =
            # (y-mu)*rstd apply + store.
            nst = max(ST // SB, 1)
            y_tiles = {}

            def emit_out_head(st):
                ps = util.tile([P, E], F32, name="o_ps", tag="u")
                for g in range(G2):
                    nc.tensor.matmul(
                        ps, lhsT=ctx_f8[g][:, :, ts(st, P)], rhs=wo_sb[:, g, :, :],
                        start=(g == 0), stop=(g == G2 - 1),
                        perf_mode=mybir.MatmulPerfMode.DoubleRow,
                    )
                xo_t = youtp.tile([P, E], F32, name="xo_t", tag="xo")
                nc.sync.dma_start(xo_t, xo_f32[ts(st, P), :])
                y_t = youtp.tile([P, E], F32, name=f"y_{st}", tag=f"y_{st}")
                nc.vector.tensor_add(out=y_t, in0=ps, in1=xo_t)
                y_tiles[st] = y_t
                stats = normp.tile([P, 6], F32, name="stats")
                nc.vector.bn_stats(out=stats, in_=y_t)
                sb, k = divmod(st, nst) if SB > 1 else (0, st)
                nc.vector.bn_aggr(out=mv8[sb][:, 2 * k : 2 * k + 2], in_=stats)

            # f32 whose bit pattern is the quake rsqrt magic 0x5f3759df
            qmagic = singles.tile([P, nst], F32, name="qmagic")
            nc.vector.memset(qmagic, 1.3211836172961054e19)

            def emit_rsqrt(sb):
                # rstd8[sb][:, k] = 1/sqrt(var_k + eps) via quake bit-trick +
                # 2 Newton steps, entirely on DVE over tiny [P, nst] tiles
                var = mv8[sb][:, 1 : 2 * nst : 2]
                v8 = normp.tile([P, nst], F32, name="q_v8", tag="qk")
                nc.vector.tensor_scalar(out=v8, in0=var, scalar1=1e-5,
                                        scalar2=None, op0=ALU.add)
                # y0 bits = magic - (v >> 1)
                sh = normp.tile([P, nst], mybir.dt.int32, name="q_sh", tag="qs")
                nc.vector.tensor_scalar(
                    out=sh, in0=v8.bitcast(mybir.dt.int32),
                    scalar1=1, scalar2=None, op0=ALU.logical_shift_right,
                )
                nc.vector.tensor_tensor(
                    out=sh, in0=qmagic.bitcast(mybir.dt.int32), in1=sh,
                    op=ALU.subtract,
                )
                y = sh.bitcast(F32)
                h_t = normp.tile([P, nst], F32, name="q_h", tag="qk2")
                nc.vector.tensor_scalar(out=h_t, in0=v8, scalar1=0.5,
                                        scalar2=None, op0=ALU.mult)
                for _ in range(2):
                    t_t = normp.tile([P, nst], F32, name="q_t", tag="qk3")
                    nc.vector.tensor_mul(out=t_t, in0=y, in1=y)
                    nc.vector.tensor_mul(out=t_t, in0=t_t, in1=h_t)
                    nc.vector.tensor_scalar(out=t_t, in0=t_t, scalar1=-1.0,
                                            scalar2=1.5, op0=ALU.mult,
                                            op1=ALU.add)
                    nc.vector.tensor_mul(out=rstd8[sb], in0=y, in1=t_t)
                    y = rstd8[sb]

            def emit_out_tail(st):
                y_t = y_tiles.pop(st)
                sb, k = divmod(st, nst) if SB > 1 else (0, st)
                nc.vector.tensor_scalar(
                    out=y_t, in0=y_t,
                    scalar1=mv8[sb][:, 2 * k : 2 * k + 1],
                    scalar2=rstd8[sb][:, k : k + 1],
                    op0=ALU.subtract, op1=ALU.mult,
                )
                if not unit_ln:
                    nc.vector.tensor_mul(out=y_t, in0=y_t, in1=g_bc)
                    nc.vector.tensor_add(out=y_t, in0=y_t, in1=b_bc)
                nc.sync.dma_start(y_out[ts(st, P), :], y_t)


            # ---------- emission: sb-major; projections stream as fillers ----
            # prologue: just enough for attention(0, sb0) to start
            emit_k(0, 0)
            emit_q(0, 0)
            for t in range(min(4, TT)):
                emit_v(t)
            if TB > 1:
                emit_k(0, 1)

            def sched(items, ng):
                """Spread callables over groups [0, ng): dict g -> [fns]."""
                by_g = {}
                if not items:
                    return by_g
                per = max(1, (len(items) + ng - 1) // ng)
                it = iter(items)
                for g in range(ng):
                    chunk = []
                    for _ in range(per):
                        f = next(it, None)
                        if f is None:
                            break
                        chunk.append(f)
                    if chunk:
                        by_g[g] = chunk
                    else:
                        break
                return by_g

            emitted_out = set()

            def of(st):
                def run():
                    emit_out_head(st)
                    emitted_out.add(st)
                return run

            for sb in range(SB):
                for hp in range(HPAIRS):
                    fb = {}
                    if sb == 0:
                        if hp == 0:
                            # self-stream: rest of own kT two groups ahead,
                            # V pairs two pairs ahead
                            for g in range(NG):
                                fs = []
                                if g % 2 == 0 and 2 <= g // 2 + 2 < TB:
                                    fs.append(lambda tb=g // 2 + 2: emit_k(0, tb))
                                p = g + 2
                                if 2 <= p < NPAIR:
                                    fs.append(lambda t=2 * p: emit_v(t))
                                    fs.append(lambda t=2 * p + 1: emit_v(t))
                                if fs:
                                    fb[g] = fs
                            # next head-pair's first k-blocks + q at the tail
                            tail = []
                            if HPAIRS > 1:
                                for tb in range(min(2, TB)):
                                    tail.append(lambda tb=tb: emit_k(1, tb))
                                tail.append(lambda: emit_q(1, 0))
                            for i, f in enumerate(tail):
                                fb.setdefault(max(0, NG - 3) + i % 3, []).append(f)
                        else:
                            items = []
                            for tb in range(2, TB):
                                items.append(lambda hp=hp, tb=tb: emit_k(hp, tb))
                            if hp + 1 < HPAIRS:
                                for tb in range(min(2, TB)):
                                    items.append(
                                        lambda hp=hp + 1, tb=tb: emit_k(hp, tb))
                                items.append(lambda hp=hp + 1: emit_q(hp, 0))
                            elif SB > 1:
                                for h2 in range(HPAIRS):
                                    items.append(lambda h2=h2: emit_q(h2, 1))
                            fb = sched(items, NG)
                    else:
                        # sb1 pass: stream one sb0 output head per block,
                        # after the deferred norms have landed (g >= 4); the
                        # batched rsqrt + applies ride the last block
                        outs_per_block = (ST // SB + HPAIRS - 1) // HPAIRS
                        items = []
                        for k in range(outs_per_block):
                            st = hp * outs_per_block + k
                            if st < ST // SB:
                                items.append(of(st))
                        for i, f in enumerate(items):
                            fb.setdefault(min(4 + i, NG - 1), []).append(f)
                        if hp == HPAIRS - 1:
                            fb.setdefault(min(8, NG - 1), []).append(
                                lambda: emit_rsqrt(0))
                            for k in range(ST // SB):
                                fb.setdefault(min(10 + k, NG - 1), []).append(
                                    lambda st=k: emit_out_tail(st))
                    attention(hp, sb, fb,
                              norm_first=(sb > 0 and hp == 0 and SB > 1))

            while pending_norm:
                pending_norm.pop(0)()
            for st in range(ST):
                if st not in emitted_out:
                    emit_out_head(st)
            emit_rsqrt(SB - 1)
            for st in range((SB - 1) * nst, ST):
                emit_out_tail(st)

    return nc


_CACHED = {}


def _get_nc(seq=S, rows=R, zero_qk_bias=True, unit_ln=True, dve_num=1, dve_den=3):
    key = (seq, rows, zero_qk_bias, unit_ln, dve_num, dve_den)
    if key not in _CACHED:
        nc = bacc.Bacc("TRN2", target_bir_lowering=False, debug=False,
                       num_devices=N_CORES)
        build_mha(nc, seq=seq, rows=rows, zero_qk_bias=zero_qk_bias,
                  unit_ln=unit_ln, dve_num=dve_num, dve_den=dve_den)
        nc.compile()
        _CACHED[key] = nc
    return _CACHED[key]


def pack_fp8_tb(x2d, tblk=512):
    """[S, E] f32 -> [128, TB, G2, 2, tblk] fp8 with
    out[p, tb, g, i, u] = x[tb*tblk+u, (2g+i)*128+p]."""
    f8 = ml_dtypes.float8_e4m3
    s, e = x2d.shape
    tb = s // tblk
    # x.T [E, S] -> [G2, 2, 128, TB, tblk] -> [128, TB, G2, 2, tblk]
    return np.ascontiguousarray(
        np.asarray(x2d, np.float32).T
        .reshape(e // 256, 2, 128, tb, tblk)
        .transpose(2, 3, 0, 1, 4)
        .astype(f8)
    )


def packw_fp8_dr(w, scale=1.0):
    """[E, E] f32 -> [128, G2, 2, E] fp8 with out[p, g, i, e] = w[(2g+i)*128+p, e]."""
    f8 = ml_dtypes.float8_e4m3
    e_in, e_out = w.shape
    return np.ascontiguousarray(
        (np.asarray(w, np.float32) * scale)
        .reshape(e_in // 256, 2, 128, e_out)
        .transpose(2, 0, 1, 3)
        .astype(f8)
    )


def make_in_maps(inputs, zero_qk_bias, unit_ln):
    """Shard full inputs into per-core input dicts."""
    x = np.asarray(inputs["x"], np.float32)
    bo = np.asarray(inputs["bo"], np.float32)
    shared = {
        "wq_f8": packw_fp8_dr(inputs["wq"]),
        "wk_f8": packw_fp8_dr(inputs["wk"], scale=BITS_MUL),
        "wv_f8": packw_fp8_dr(inputs["wv"]),
        "wo_f8": packw_fp8_dr(inputs["wo"]),
        "bv": np.asarray(inputs["bv"], np.float32),
    }
    if not zero_qk_bias:
        shared["bq"] = np.asarray(inputs["bq"], np.float32)
        shared["bk"] = np.asarray(inputs["bk"], np.float32) * BITS_MUL
    if not unit_ln:
        shared["ln_g"] = np.asarray(inputs["ln_g"], np.float32)
        shared["ln_b"] = np.asarray(inputs["ln_b"], np.float32)
    xT_all = [pack_fp8_tb(x[b]) for b in range(B)]
    in_maps = []
    for c in range(N_CORES):
        b, shard = divmod(c, SEQ_SHARDS)
        r0 = shard * R
        m = dict(shared)
        m["xT_f8"] = xT_all[b]
        m["xoT_f8"] = pack_fp8_tb(x[b, r0 : r0 + R])
        m["xo_f32"] = np.ascontiguousarray(x[b, r0 : r0 + R] + bo)
        in_maps.append(m)
    return in_maps


def kernel(**inputs):
    from concourse import bass_utils

    zero_qk_bias = (not np.any(inputs["bq"])) and (not np.any(inputs["bk"]))
    unit_ln = bool(np.all(np.asarray(inputs["ln_g"]) == 1.0)) and (
        not np.any(inputs["ln_b"]))
    nc = _get_nc(zero_qk_bias=zero_qk_bias, unit_ln=unit_ln)
    in_maps = make_in_maps(inputs, zero_qk_bias, unit_ln)
    res = bass_utils.run_bass_kernel_spmd(nc, in_maps, core_ids=list(range(N_CORES)))
    out = np.empty((B, S, E), np.float32)
    for c in range(N_CORES):
        b, shard = divmod(c, SEQ_SHARDS)
        out[b, shard * R : (shard + 1) * R] = res.results[c]["y"]
    return out


# revision 22
# speedup vs baseline: 1.3168x; 1.1405x over previous
"""Trainium2 Bass kernel for a full MHA block (QKV proj + softmax attention +
output proj + residual + LayerNorm), B=2, S=4096, E=512, H=8, D=64.

Sharding: sequence-parallel over 8 cores (4 seq shards x 2 batches). Each core
owns R=1024 query rows of one batch, recomputes K/V for the full context
(avoids all cross-core communication), and writes its own [R, E] output slice.

v2 layout/scheduling strategy (per core):
  - x^T pre-transposed+packed fp8 on host, t-blocked so DMA loads are
    contiguous 2KB/partition chunks spread over 4 engine queues
  - K^T/Q^T projections head-major [e_out/128, t] (fp8 DoubleRow, K=256)
  - wk pre-scaled by BITS_MUL on host so the DVE exp bit-trick is a
    single-op tensor_scalar add (2x mode) and scores arrive pre-scaled
  - scores: per t-tile the two heads of a pair are issued back-to-back at
    tile_position (0,0)/(64,0) so the K=64 matmuls run concurrently in
    separate PE row groups; A@V of the previous group follows them
  - exp split ~50/50 between ScalarE (true exp, scale folds the prescale)
    and DVE (log-domain fp8 bit trick)
  - A@V: lhsT = [V_h | ones] (80 cols, fp8 DoubleRow K=256); row 64
    accumulates the softmax denominator for free
  - normalize: batched reciprocal of the two denom rows, partition-broadcast
    on GpSimd, DVE multiply writing ctx^T directly in fp8 DoubleRow-packed
    layout for the O-projection
  - O-proj: 2 fp8 DoubleRow matmuls (K=256 each, all 8 heads) + residual
    (bo pre-folded into the residual input on host) + LayerNorm with
    rstd = exp(-0.5*ln(var+eps)) so ScalarE stays on one activation-table
    set (no Exp<->Sqrt table thrash)
"""

import sys

sys.path.insert(0, "/opt/trn_rl_repo")

import numpy as np
import ml_dtypes

import concourse.bass as bass
import concourse.bacc as bacc
import concourse.mybir as mybir
import concourse.tile as tile
from concourse.bass import ds, ts

# Problem constants (hardcoded per harness contract)
B = 2
S = 4096
E = 512
H = 8
D = 64
N_CORES = 8
SEQ_SHARDS = N_CORES // B
R = S // SEQ_SHARDS  # 1024 own query rows per core
G2 = E // 256        # DoubleRow chunk-pair groups for the projections

F32 = mybir.dt.float32
F16 = mybir.dt.float16
FP8 = mybir.dt.float8e4
VP = 80  # padded V columns (64 V + 1 ones + pad to a 16-multiple for DoubleRow)
EXP_SHIFT = -3.0  # exp(s/8 - 3): keeps exp outputs < fp8e4 max; cancels in softmax
# log-domain exp on DVE: fp8e4m3 bits of exp(s/8+SHIFT) == s*BITS_MUL + BITS_ADD,
# rounded + saturated to [0,255] by the uint8 convert (verified on HW).
# wk is pre-scaled by BITS_MUL on host, so scores arrive as s' = s*BITS_MUL and
# the DVE op is a single add; the scalar path divides the scale back out.
BITS_MUL = 11.5416529 / 8.0
BITS_ADD = 56.0 + 11.5416529 * EXP_SHIFT
SCALAR_SCALE = 1.0 / 11.5416529  # exp(s'/11.5416529 + SHIFT) == exp(s/8 + SHIFT)
AF = mybir.ActivationFunctionType
ALU = mybir.AluOpType


def build_mha(nc, seq=S, rows=R, zero_qk_bias=True, unit_ln=True,
              dve_num=1, dve_den=2):
    """Emit the Tile program. seq/rows shrinkable for simulation."""
    P = 128
    EC = E // P           # 4 e_out col blocks
    HPAIRS = H // 2       # 4 head-pair blocks (=e_out blocks of 128)
    TT = seq // P         # t tiles
    NPAIR = TT // 2       # t-tile pairs (DoubleRow A@V granularity)
    tblk = min(512, seq)
    TB = seq // tblk      # t blocks for xT DMA / K-proj
    tpb = tblk // P       # t tiles per block
    qblk = min(512, rows)
    QB = rows // qblk     # r blocks for Q-proj
    sblk = min(512, rows)
    SB = rows // sblk     # s blocks per core
    ST = rows // P        # s tiles for O-proj/LN
    NG = NPAIR            # score groups (one per t-tile pair)

    # ---- DRAM I/O ----
    # x fed pre-transposed+packed fp8 from host, t-blocked:
    #   xT[p, tb, g, i, u] = x[tb*tblk+u, (2g+i)*128+p]   (DoubleRow pair axis i)
    xT_d = nc.dram_tensor("xT_f8", [P, TB, G2, 2, tblk], FP8, kind="ExternalInput").ap()
    xoT_d = nc.dram_tensor("xoT_f8", [P, QB, G2, 2, qblk], FP8, kind="ExternalInput").ap()
    # residual rows with bo pre-folded on host
    xo_f32 = nc.dram_tensor("xo_f32", [rows, E], F32, kind="ExternalInput").ap()
    # weights pre-packed on host: wX[p, g, i, e] = w[(2g+i)*128+p, e]
    # (wk additionally pre-scaled by BITS_MUL)
    wq = nc.dram_tensor("wq_f8", [P, G2, 2, E], FP8, kind="ExternalInput").ap()
    wk = nc.dram_tensor("wk_f8", [P, G2, 2, E], FP8, kind="ExternalInput").ap()
    wv = nc.dram_tensor("wv_f8", [P, G2, 2, E], FP8, kind="ExternalInput").ap()
    wo = nc.dram_tensor("wo_f8", [P, G2, 2, E], FP8, kind="ExternalInput").ap()
    bv = nc.dram_tensor("bv", [E], F32, kind="ExternalInput").ap()
    if not zero_qk_bias:
        # host pre-scales bk by BITS_MUL to match the wk prescale
        bq = nc.dram_tensor("bq", [E], F32, kind="ExternalInput").ap()
        bk = nc.dram_tensor("bk", [E], F32, kind="ExternalInput").ap()
    if not unit_ln:
        ln_g = nc.dram_tensor("ln_g", [E], F32, kind="ExternalInput").ap()
        ln_b = nc.dram_tensor("ln_b", [E], F32, kind="ExternalInput").ap()
    y_out = nc.dram_tensor("y", [rows, E], F32, kind="ExternalOutput").ap()

    with tile.TileContext(nc) as tc:
        with (
            tc.tile_pool(name="singles", bufs=1) as singles,
            tc.tile_pool(name="kqv", bufs=1) as kqv,
            tc.tile_pool(name="vtiles", bufs=max(NPAIR, 2)) as vtiles,
            tc.tile_pool(name="at", bufs=4) as atp,
            tc.tile_pool(name="ctx", bufs=4) as ctxp,
            tc.tile_pool(name="norm", bufs=4) as normp,
            tc.tile_pool(name="yout", bufs=3) as youtp,
            tc.tile_pool(name="stg", bufs=2, space="PSUM") as stg,
            tc.tile_pool(name="acc", bufs=2, space="PSUM") as accp,
            tc.tile_pool(name="util", bufs=2, space="PSUM") as util,
        ):
            # ---------- weights / x^T loads (4 DMA queues, startup-critical
            # order: what emit_k(0,0)/emit_q(0,0)/emit_v(0..) need comes first)
            wq_sb = singles.tile([P, G2, 2, E], FP8, name="wq_sb")
            wk_sb = singles.tile([P, G2, 2, E], FP8, name="wk_sb")
            wv_sb = singles.tile([P, G2, 2, E], FP8, name="wv_sb")
            wo_sb = singles.tile([P, G2, 2, E], FP8, name="wo_sb")
            xT = singles.tile([P, TB, G2, 2, tblk], FP8, name="xT")
            xoT = singles.tile([P, QB, G2, 2, qblk], FP8, name="xoT")
            nc.sync.dma_start(xT[:, 0], xT_d[:, 0])
            nc.gpsimd.dma_start(wk_sb, wk)
            nc.scalar.dma_start(wq_sb, wq)
            nc.scalar.dma_start(xoT[:, 0], xoT_d[:, 0])
            nc.gpsimd.dma_start(wv_sb, wv)
            if TB > 1:
                nc.sync.dma_start(xT[:, 1], xT_d[:, 1])
            for rb in range(1, QB):
                nc.scalar.dma_start(xoT[:, rb], xoT_d[:, rb])
            qrot = [nc.sync, nc.gpsimd, nc.scalar]
            for tb in range(2, TB):
                qrot[tb % 3].dma_start(xT[:, tb], xT_d[:, tb])
            nc.gpsimd.dma_start(wo_sb, wo)

            # ---------- constants ----------
            bv_bc = singles.tile([P, E], F32, name="bv_bc")
            nc.gpsimd.dma_start(out=bv_bc, in_=bv[None, :].to_broadcast((P, E)))
            if not zero_qk_bias:
                bk_sb = singles.tile([P, EC], F32, name="bk_sb")
                bq_sb = singles.tile([P, EC], F32, name="bq_sb")
                nc.gpsimd.dma_start(bk_sb, bk.rearrange("(c p) -> p c", p=P))
                nc.gpsimd.dma_start(bq_sb, bq.rearrange("(c p) -> p c", p=P))
            if not unit_ln:
                g_bc = singles.tile([P, E], F32, name="g_bc")
                b_bc = singles.tile([P, E], F32, name="b_bc")
                nc.gpsimd.dma_start(out=g_bc, in_=ln_g[None, :].to_broadcast((P, E)))
                nc.gpsimd.dma_start(out=b_bc, in_=ln_b[None, :].to_broadcast((P, E)))
            shift_t = singles.tile([P, 1], F32, name="shift_t")
            nc.vector.memset(shift_t, EXP_SHIFT)
            # per-sb LayerNorm stats: bn_aggr lands mean/var pairs here so the
            # rsqrt can be batched on DVE (no ScalarE act-table thrash)
            mv8 = [singles.tile([P, 2 * max(ST // SB, 1)], F32, name=f"mv8_{sb}")
                   for sb in range(SB)]
            rstd8 = [singles.tile([P, max(ST // SB, 1)], F32, name=f"rstd8_{sb}")
                     for sb in range(SB)]
            # pre-warm the GpSimd ext-isa library for partition_broadcast (the
            # first call otherwise pays a ~7us IRAM library DMA mid-kernel)
            warm_in = singles.tile([1, 16], F32, name="warm_in")
            warm_out = singles.tile([D, 16], F32, name="warm_out")
            nc.vector.memset(warm_in, 1.0)
            nc.gpsimd.partition_broadcast(warm_out, warm_in)

            # ---------- projection targets ----------
            kT = [kqv.tile([P, seq], F16, name=f"kT_{hp}") for hp in range(HPAIRS)]
            qT = [kqv.tile([P, rows], F16, name=f"qT_{hp}") for hp in range(HPAIRS)]
            # ctx^T in fp8 DoubleRow-packed layout for the O-projection:
            # ctx_f8[g][p, i, s] = ctx[head=(256g+128i+p)//64, d=p%64, s] / denom
            ctx_f8 = [kqv.tile([P, 2, rows], FP8, name=f"ctxf8_{g}")
                      for g in range(G2)]

            # ---------- V projection (+bias, +ones col) ----------
            v_tiles = {}

            def emit_v(t):
                pair, i = divmod(t, 2)
                if i == 0:
                    vt = vtiles.tile([P, 2, H, VP], FP8, name=f"v_{pair}", tag="v")
                    nc.vector.memset(vt[:, :, :, D:VP], 0.0)
                    nc.vector.memset(vt[:, :, :, D : D + 1], 1.0)
                    v_tiles[pair] = vt
                vt = v_tiles[pair]
                ps = util.tile([P, E], F32, name="v_ps", tag="u")
                tb, u = divmod(t, tpb)
                for g in range(G2):
                    nc.tensor.matmul(
                        ps, lhsT=xT[:, tb, g, :, ds(u * P, P)], rhs=wv_sb[:, g, :, :],
                        start=(g == 0), stop=(g == G2 - 1),
                        perf_mode=mybir.MatmulPerfMode.DoubleRow,
                    )
                nc.vector.tensor_add(
                    out=vt[:, i, :, 0:D],
                    in0=ps.rearrange("p (h d) -> p h d", h=H),
                    in1=bv_bc.rearrange("p (h d) -> p h d", h=H),
                )

            # ---------- K^T / Q^T projections (per head-pair block) ----------
            def emit_k(hp, tb):
                ps = util.tile([P, 512], F32, name="k_ps", tag="u")
                for g in range(G2):
                    nc.tensor.matmul(
                        ps[:, :tblk], lhsT=wk_sb[:, g, :, ds(hp * P, P)],
                        rhs=xT[:, tb, g, :, :],
                        start=(g == 0), stop=(g == G2 - 1),
                        perf_mode=mybir.MatmulPerfMode.DoubleRow,
                    )
                dst = kT[hp][:, ds(tb * tblk, tblk)]
                if zero_qk_bias:
                    nc.vector.tensor_copy(dst, ps[:, :tblk])
                else:
                    nc.vector.tensor_scalar(
                        out=dst, in0=ps[:, :tblk],
                        scalar1=bk_sb[:, hp : hp + 1], scalar2=None,
                        op0=ALU.add,
                    )

            def emit_q(hp, rb):
                ps = util.tile([P, 512], F32, name="q_ps", tag="u")
                for g in range(G2):
                    nc.tensor.matmul(
                        ps[:, :qblk], lhsT=wq_sb[:, g, :, ds(hp * P, P)],
                        rhs=xoT[:, rb, g, :, :],
                        start=(g == 0), stop=(g == G2 - 1),
                        perf_mode=mybir.MatmulPerfMode.DoubleRow,
                    )
                dst = qT[hp][:, ds(rb * qblk, qblk)]
                if zero_qk_bias:
                    nc.vector.tensor_copy(dst, ps[:, :qblk])
                else:
                    nc.vector.tensor_scalar(
                        out=dst, in0=ps[:, :qblk],
                        scalar1=bq_sb[:, hp : hp + 1], scalar2=None,
                        op0=ALU.add,
                    )

            # ---------- attention ----------
            exp_ctr = [0]
            # deferred normalize closures (see v1): each block's tail runs a
            # few groups into the NEXT block so the PE queue never stalls on
            # the drain/reciprocal chain at block boundaries.
            pending_norm = []

            def attention(hp, sb, fillers_by_group, norm_first=False):
                if norm_first and pending_norm:
                    pending_norm.pop(0)()
                ctx_ps = [
                    accp.tile([VP, sblk], F32, name=f"ctx_{h}", tag="ctx")
                    for h in range(2)
                ]
                pending = []  # at-pairs awaiting A@V, one group behind

                def flush_av(last):
                    at_p, pair = pending.pop(0)
                    for h in range(2):
                        nc.tensor.matmul(
                            ctx_ps[h][:, :sblk],
                            lhsT=v_tiles[pair][:, :, hp * 2 + h, :],
                            rhs=at_p[h][:, :, :sblk],
                            start=(pair == 0), stop=last,
                            perf_mode=mybir.MatmulPerfMode.DoubleRow,
                        )

                consumed = set()
                for g in range(NG):
                    if g == 2 and not norm_first and pending_norm:
                        pending_norm.pop(0)()
                    st_pair = [
                        stg.tile([P, 2, 512], F32, name=f"stg_{h}", tag="stg")
                        for h in range(2)
                    ]
                    at_pair = [
                        atp.tile([P, 2, 512], FP8, name=f"at_{h}", tag="at")
                        for h in range(2)
                    ]
                    # scores: heads issued adjacently at row groups (0,0)/(64,0)
                    # so the two K=64 matmuls stream concurrently
                    for j in range(2):
                        t = 2 * g + j
                        for h in range(2):
                            nc.tensor.matmul(
                                st_pair[h][:, j, :sblk],
                                lhsT=kT[hp][ds(h * D, D), ts(t, P)],
                                rhs=qT[hp][ds(h * D, D), ds(sb * sblk, sblk)],
                                start=True, stop=True,
                                tile_position=(h * D, 0),
                            )
                    # A@V of the previous group runs while exp(g) processes
                    if pending:
                        flush_av(False)
                    # exp: split between DVE (log-domain bit trick: single add,
                    # wk prescale folded the multiply) and ScalarE (true exp);
                    # both are capped ~1x by the PSUM read port
                    for h in range(2):
                        if (exp_ctr[0] * dve_num) % dve_den < dve_num:
                            nc.vector.tensor_scalar(
                                out=at_pair[h][:, :, :sblk].bitcast(mybir.dt.uint8),
                                in0=st_pair[h][:, :, :sblk],
                                scalar1=BITS_ADD, scalar2=None,
                                op0=ALU.add,
                            )
                        else:
                            nc.scalar.activation(
                                out=at_pair[h][:, :, :sblk],
                                in_=st_pair[h][:, :, :sblk],
                                func=AF.Exp, scale=SCALAR_SCALE, bias=shift_t,
                            )
                        exp_ctr[0] += 1
                    pending.append((at_pair, g))
                    consumed.add(g)
                    for f in fillers_by_group.get(g, ()):
                        f()
                # run any fillers scheduled past the last group (small configs)
                for g_key in sorted(k for k in fillers_by_group if k not in consumed):
                    for f in fillers_by_group[g_key]:
                        f()
                flush_av(True)
                # drain ctx+denominator rows PSUM->SBUF on DVE (DMA cannot
                # touch PSUM); denom rows hop to partitions 0/1 by SBUF DMA
                ctx_sb = [ctxp.tile([D + 1, sblk], F32, name=f"cs_{h}", tag="cs")
                          for h in range(2)]
                den2 = normp.tile([2, sblk], F32, name="den2", tag="dn")
                for h in range(2):
                    nc.vector.tensor_copy(ctx_sb[h], ctx_ps[h][: D + 1, :sblk])
                    nc.sync.dma_start(den2[h : h + 1, :], ctx_sb[h][D : D + 1, :])

                def do_norm(hp=hp, sb=sb, ctx_sb=ctx_sb, den2=den2):
                    recip2 = normp.tile([2, sblk], F32, name="recip2", tag="rc")
                    nc.vector.reciprocal_approx_fast(out=recip2, in_=den2)
                    # partition_broadcast sources must sit at partition 0:
                    # hop row 1 down via SBUF DMA (off the critical path)
                    r1 = normp.tile([1, sblk], F32, name="recip_r1", tag="r1")
                    nc.gpsimd.dma_start(r1, recip2[1:2, :])
                    for h in range(2):
                        rb_t = normp.tile([D, sblk], F32, name=f"rb_{h}", tag="rb")
                        nc.gpsimd.partition_broadcast(
                            rb_t, recip2[0:1, :] if h == 0 else r1)
                        head = hp * 2 + h
                        gi, ii, plo = head // 4, (head % 4) // 2, D * (head % 2)
                        nc.vector.tensor_mul(
                            out=ctx_f8[gi][ds(plo, D), ii, ds(sb * sblk, sblk)],
                            in0=ctx_sb[h][0:D, :], in1=rb_t,
                        )

                pending_norm.append(do_norm)

            # ---------- O-projection + residual + LayerNorm ----------
            # split: head = O-proj + residual + bn stats (streamable during
            # attention); rsqrt = one batched DVE quake-rsqrt per sb (keeps
            # ScalarE on the exp table set -- no act-table thrash); tail =
            # (y-mu)*rstd apply + store.
            nst = max(ST // SB, 1)
            y_tiles = {}

            def emit_out_head(st):
                ps = util.tile([P, E], F32, name="o_ps", tag="u")
                for g in range(G2):
                    nc.tensor.matmul(
                        ps, lhsT=ctx_f8[g][:, :, ts(st, P)], rhs=wo_sb[:, g, :, :],
                        start=(g == 0), stop=(g == G2 - 1),
                        perf_mode=mybir.MatmulPerfMode.DoubleRow,
                    )
                xo_t = youtp.tile([P, E], F32, name="xo_t", tag="xo")
                nc.sync.dma_start(xo_t, xo_f32[ts(st, P), :])
                y_t = youtp.tile([P, E], F32, name=f"y_{st}", tag=f"y_{st}")
                nc.vector.tensor_add(out=y_t, in0=ps, in1=xo_t)
                y_tiles[st] = y_t
                stats = normp.tile([P, 6], F32, name="stats")
                nc.vector.bn_stats(out=stats, in_=y_t)
                sb, k = divmod(st, nst) if SB > 1 else (0, st)
                nc.vector.bn_aggr(out=mv8[sb][:, 2 * k : 2 * k + 2], in_=stats)

            # f32 whose bit pattern is the quake rsqrt magic 0x5f3759df
            qmagic = singles.tile([P, nst], F32, name="qmagic")
            nc.vector.memset(qmagic, 1.3211836172961054e19)

            def emit_rsqrt(sb):
                # rstd8[sb][:, k] = 1/sqrt(var_k + eps) via quake bit-trick +
                # 2 Newton steps, entirely on DVE over tiny [P, nst] tiles
                var = mv8[sb][:, 1 : 2 * nst : 2]
                v8 = normp.tile([P, nst], F32, name="q_v8", tag="qk")
                nc.vector.tensor_scalar(out=v8, in0=var, scalar1=1e-5,
                                        scalar2=None, op0=ALU.add)
                # y0 bits = magic - (v >> 1)
                sh = normp.tile([P, nst], mybir.dt.int32, name="q_sh", tag="qs")
                nc.vector.tensor_scalar(
                    out=sh, in0=v8.bitcast(mybir.dt.int32),
                    scalar1=1, scalar2=None, op0=ALU.logical_shift_right,
                )
                nc.vector.tensor_tensor(
                    out=sh, in0=qmagic.bitcast(mybir.dt.int32), in1=sh,
                    op=ALU.subtract,
                )
                y = sh.bitcast(F32)
                h_t = normp.tile([P, nst], F32, name="q_h", tag="qk2")
                nc.vector.tensor_scalar(out=h_t, in0=v8, scalar1=0.5,
                                        scalar2=None, op0=ALU.mult)
                for _ in range(2):
                    t_t = normp.tile([P, nst], F32, name="q_t", tag="qk3")
                    nc.vector.tensor_mul(out=t_t, in0=y, in1=y)
                    nc.vector.tensor_mul(out=t_t, in0=t_t, in1=h_t)
                    nc.gpsimd.tensor_scalar(out=t_t, in0=t_t, scalar1=-1.0,
                                            scalar2=1.5, op0=ALU.mult,
                                            op1=ALU.add)
                    nc.vector.tensor_mul(out=rstd8[sb], in0=y, in1=t_t)
                    y = rstd8[sb]

            def emit_out_tail(st):
                y_t = y_tiles.pop(st)
                sb, k = divmod(st, nst) if SB > 1 else (0, st)
                nc.vector.tensor_scalar(
                    out=y_t, in0=y_t,
                    scalar1=mv8[sb][:, 2 * k : 2 * k + 1],
                    scalar2=rstd8[sb][:, k : k + 1],
                    op0=ALU.subtract, op1=ALU.mult,
                )
                if not unit_ln:
                    nc.vector.tensor_mul(out=y_t, in0=y_t, in1=g_bc)
                    nc.vector.tensor_add(out=y_t, in0=y_t, in1=b_bc)
                nc.sync.dma_start(y_out[ts(st, P), :], y_t)


            # ---------- emission: sb-major; projections stream as fillers ----
            # prologue: just enough for attention(0, sb0) to start
            emit_k(0, 0)
            emit_q(0, 0)
            for t in range(min(4, TT)):
                emit_v(t)
            if TB > 1:
                emit_k(0, 1)

            def sched(items, ng):
                """Spread callables over groups [0, ng): dict g -> [fns]."""
                by_g = {}
                if not items:
                    return by_g
                per = max(1, (len(items) + ng - 1) // ng)
                it = iter(items)
                for g in range(ng):
                    chunk = []
                    for _ in range(per):
                        f = next(it, None)
                        if f is None:
                            break
                        chunk.append(f)
                    if chunk:
                        by_g[g] = chunk
                    else:
                        break
                return by_g

            emitted_out = set()

            def of(st):
                def run():
                    emit_out_head(st)
                    emitted_out.add(st)
                return run

            for sb in range(SB):
                for hp in range(HPAIRS):
                    fb = {}
                    if sb == 0:
                        if hp == 0:
                            # self-stream: rest of own kT two groups ahead,
                            # V pairs two pairs ahead
                            for g in range(NG):
                                fs = []
                                if g % 2 == 0 and 2 <= g // 2 + 2 < TB:
                                    fs.append(lambda tb=g // 2 + 2: emit_k(0, tb))
                                p = g + 2
                                if 2 <= p < NPAIR:
                                    fs.append(lambda t=2 * p: emit_v(t))
                                    fs.append(lambda t=2 * p + 1: emit_v(t))
                                if fs:
                                    fb[g] = fs
                            # next head-pair's first k-blocks + q at the tail
                            tail = []
                            if HPAIRS > 1:
                                for tb in range(min(2, TB)):
                                    tail.append(lambda tb=tb: emit_k(1, tb))
                                tail.append(lambda: emit_q(1, 0))
                            for i, f in enumerate(tail):
                                fb.setdefault(max(0, NG - 3) + i % 3, []).append(f)
                        else:
                            items = []
                            for tb in range(2, TB):
                                items.append(lambda hp=hp, tb=tb: emit_k(hp, tb))
                            if hp + 1 < HPAIRS:
                                for tb in range(min(2, TB)):
                                    items.append(
                                        lambda hp=hp + 1, tb=tb: emit_k(hp, tb))
                                items.append(lambda hp=hp + 1: emit_q(hp, 0))
                            elif SB > 1:
                                for h2 in range(HPAIRS):
                                    items.append(lambda h2=h2: emit_q(h2, 1))
                            fb = sched(items, NG)
                    else:
                        # sb1 pass: stream one sb0 output head per block,
                        # after the deferred norms have landed (g >= 4); the
                        # batched rsqrt + applies ride the last block
                        outs_per_block = (ST // SB + HPAIRS - 1) // HPAIRS
                        items = []
                        for k in range(outs_per_block):
                            st = hp * outs_per_block + k
                            if st < ST // SB:
                                items.append(of(st))
                        for i, f in enumerate(items):
                            fb.setdefault(min(4 + i, NG - 1), []).append(f)
                        if hp == HPAIRS - 1:
                            fb.setdefault(min(8, NG - 1), []).append(
                                lambda: emit_rsqrt(0))
                            for k in range(ST // SB):
                                fb.setdefault(min(10 + k, NG - 1), []).append(
                                    lambda st=k: emit_out_tail(st))
                    attention(hp, sb, fb,
                              norm_first=(sb > 0 and hp == 0 and SB > 1))

            while pending_norm:
                pending_norm.pop(0)()
            for st in range(ST):
                if st not in emitted_out:
                    emit_out_head(st)
            emit_rsqrt(SB - 1)
            for st in range((SB - 1) * nst, ST):
                emit_out_tail(st)

    return nc


_CACHED = {}


def _get_nc(seq=S, rows=R, zero_qk_bias=True, unit_ln=True, dve_num=1, dve_den=2):
    key = (seq, rows, zero_qk_bias, unit_ln, dve_num, dve_den)
    if key not in _CACHED:
        nc = bacc.Bacc("TRN2", target_bir_lowering=False, debug=False,
                       num_devices=N_CORES)
        build_mha(nc, seq=seq, rows=rows, zero_qk_bias=zero_qk_bias,
                  unit_ln=unit_ln, dve_num=dve_num, dve_den=dve_den)
        nc.compile()
        _CACHED[key] = nc
    return _CACHED[key]


def pack_fp8_tb(x2d, tblk=512):
    """[S, E] f32 -> [128, TB, G2, 2, tblk] fp8 with
    out[p, tb, g, i, u] = x[tb*tblk+u, (2g+i)*128+p]."""
    f8 = ml_dtypes.float8_e4m3
    s, e = x2d.shape
    tb = s // tblk
    # x.T [E, S] -> [G2, 2, 128, TB, tblk] -> [128, TB, G2, 2, tblk]
    return np.ascontiguousarray(
        np.asarray(x2d, np.float32).T
        .reshape(e // 256, 2, 128, tb, tblk)
        .transpose(2, 3, 0, 1, 4)
        .astype(f8)
    )


def packw_fp8_dr(w, scale=1.0):
    """[E, E] f32 -> [128, G2, 2, E] fp8 with out[p, g, i, e] = w[(2g+i)*128+p, e]."""
    f8 = ml_dtypes.float8_e4m3
    e_in, e_out = w.shape
    return np.ascontiguousarray(
        (np.asarray(w, np.float32) * scale)
        .reshape(e_in // 256, 2, 128, e_out)
        .transpose(2, 0, 1, 3)
        .astype(f8)
    )


def make_in_maps(inputs, zero_qk_bias, unit_ln):
    """Shard full inputs into per-core input dicts."""
    x = np.asarray(inputs["x"], np.float32)
    bo = np.asarray(inputs["bo"], np.float32)
    shared = {
        "wq_f8": packw_fp8_dr(inputs["wq"]),
        "wk_f8": packw_fp8_dr(inputs["wk"], scale=BITS_MUL),
        "wv_f8": packw_fp8_dr(inputs["wv"]),
        "wo_f8": packw_fp8_dr(inputs["wo"]),
        "bv": np.asarray(inputs["bv"], np.float32),
    }
    if not zero_qk_bias:
        shared["bq"] = np.asarray(inputs["bq"], np.float32)
        shared["bk"] = np.asarray(inputs["bk"], np.float32) * BITS_MUL
    if not unit_ln:
        shared["ln_g"] = np.asarray(inputs["ln_g"], np.float32)
        shared["ln_b"] = np.asarray(inputs["ln_b"], np.float32)
    xT_all = [pack_fp8_tb(x[b]) for b in range(B)]
    in_maps = []
    for c in range(N_CORES):
        b, shard = divmod(c, SEQ_SHARDS)
        r0 = shard * R
        m = dict(shared)
        m["xT_f8"] = xT_all[b]
        m["xoT_f8"] = pack_fp8_tb(x[b, r0 : r0 + R])
        m["xo_f32"] = np.ascontiguousarray(x[b, r0 : r0 + R] + bo)
        in_maps.append(m)
    return in_maps


def kernel(**inputs):
    from concourse import bass_utils

    zero_qk_bias = (not np.any(inputs["bq"])) and (not np.any(inputs["bk"]))
    unit_ln = bool(np.all(np.asarray(inputs["ln_g"]) == 1.0)) and (
        not np.any(inputs["ln_b"]))
    nc = _get_nc(zero_qk_bias=zero_qk_bias, unit_ln=unit_ln)
    in_maps = make_in_maps(inputs, zero_qk_bias, unit_ln)
    res = bass_utils.run_bass_kernel_spmd(nc, in_maps, core_ids=list(range(N_CORES)))
    out = np.empty((B, S, E), np.float32)
    for c in range(N_CORES):
        b, shard = divmod(c, SEQ_SHARDS)
        out[b, shard * R : (shard + 1) * R] = res.results[c]["y"]
    return out
